# revision 26
# baseline (speedup 1.0000x reference)
"""Fused multi-head causal+padding attention for Trainium2 (Bass/Tile).

Problem: nn_Attention (B=8, T=1024, C=512, H=8, D=64, TT=4), f32.
Sharding: data-parallel over batch B across 8 NeuronCores (1 batch elem/core).

Per-core pipeline (batch b, everything stays on-chip between HBM load/store):
  1. x^T via PE transposes (needed so q/k come out in [d, t] layout).
  2. qk^T = W_qk^T @ x^T  (heads in [d, t] layout -> no transposes in attention)
     v    = x @ W_v       (standard [t, d] layout -> AV lhsT)
  3. per head: S^T[k,q] = k^T.T@q^T ; exp on ACT; multiplicative mask (bf16);
     y^T[d,q] (+ denominator row via an appended ones column on v) on PE;
     per-column normalize via reciprocal + partition_broadcast.
  4. out = y @ W_proj + b_eff (bias preloaded into PSUM via a K=1 matmul).

Host-side prep folds the 1/sqrt(D) scale into W_q/b_q and folds
b_v @ W_proj + b_proj into a single effective output bias.
"""

import numpy as np
import ml_dtypes
from contextlib import ExitStack

B, T, C, H, TT = 8, 1024, 512, 8, 4
D = C // H
NCORES = 8

_CACHE = {}


def _build_nc(reps=1, upto=4):
    import concourse.bass as bass
    import concourse.mybir as mybir
    import concourse.tile as tile
    from concourse import bacc
    from concourse.bass import ts
    from concourse.masks import make_identity

    dt = mybir.dt
    F32, F32R, BF16 = dt.float32, dt.float32r, dt.bfloat16
    AF = mybir.ActivationFunctionType

    nc = bacc.Bacc("TRN2", target_bir_lowering=False, debug=False,
                   num_devices=NCORES)

    x_d = nc.dram_tensor("x", [T, C], F32, kind="ExternalInput").ap()
    wqk_d = nc.dram_tensor("wqk", [C, 2 * C], F32R, kind="ExternalInput").ap()
    wv_d = nc.dram_tensor("wv", [C, C], F32R, kind="ExternalInput").ap()
    wp_d = nc.dram_tensor("wp", [C, C], F32R, kind="ExternalInput").ap()
    bqk_d = nc.dram_tensor("bqk", [2 * C], F32, kind="ExternalInput").ap()
    beff_d = nc.dram_tensor("beff", [1, C], F32R, kind="ExternalInput").ap()
    ones_d = nc.dram_tensor("ones1", [1, 128], F32R, kind="ExternalInput").ap()
    mask_d = nc.dram_tensor("maskT", [T, T], BF16, kind="ExternalInput").ap()
    out_d = nc.dram_tensor("out", [T, C], F32, kind="ExternalOutput").ap()

    TK = T // 128   # 8 tiles of 128 along t
    CK = C // 128   # 4 tiles of 128 along c

    with tile.TileContext(nc) as tc, ExitStack() as ctx:
        consts = ctx.enter_context(tc.tile_pool(name="consts", bufs=1))

        ident = consts.tile([128, 128], F32)
        make_identity(nc, ident)
        # weights on the scalar-engine HWDGE queue so the x loads (sync
        # queue) land first and compute starts immediately; mask on SWDGE.
        wqk_s = consts.tile([128, CK, 2 * C], F32R)
        nc.scalar.dma_start(out=wqk_s, in_=wqk_d.rearrange("(j p) n -> p j n", p=128))
        wv_s = consts.tile([128, CK, C], F32R)
        nc.scalar.dma_start(out=wv_s, in_=wv_d.rearrange("(j p) n -> p j n", p=128))
        wp_s = consts.tile([128, CK, C], F32R)
        nc.scalar.dma_start(out=wp_s, in_=wp_d.rearrange("(j p) n -> p j n", p=128))
        bqk_s = consts.tile([128, 2 * C // 128], F32)
        nc.gpsimd.dma_start(out=bqk_s, in_=bqk_d.rearrange("(i p) -> p i", p=128))
        beff_s = consts.tile([1, C], F32R)
        nc.scalar.dma_start(out=beff_s, in_=beff_d)
        ones1 = consts.tile([1, 128], F32R)
        nc.scalar.dma_start(out=ones1, in_=ones_d)
        mask_s = consts.tile([128, TK, T], BF16)
        nc.gpsimd.dma_start(out=mask_s, in_=mask_d.rearrange("(kt p) q -> p kt q", p=128))

        xT = consts.tile([128, CK, T], F32R)
        qkT = consts.tile([128, 2 * C // 128, T], F32R)
        vaug = consts.tile([128, TK, H, D + 1], BF16)
        yT = consts.tile([128, CK, T], F32R)

        def run_body():
            body(nc, tc, ts, F32, F32R, BF16, AF, TK, CK,
                 x_d, out_d, ident, wqk_s, wv_s, wp_s, bqk_s, beff_s, ones1,
                 mask_s, xT, qkT, vaug, yT, upto)

        if reps == 1:
            run_body()
        else:
            with tc.For_i(0, reps, 1):
                run_body()

    nc.compile()
    return nc


def body(nc, tc, ts, F32, F32R, BF16, AF, TK, CK,
         x_d, out_d, ident, wqk_s, wv_s, wp_s, bqk_s, beff_s, ones1,
         mask_s, xT, qkT, vaug, yT, upto=4):
        # ---- phase 1: load x, transpose to x^T ----
        with tc.tile_pool(name="xstage", bufs=3) as xst, \
             tc.tile_pool(name="ptr", bufs=4, space="PSUM") as ptr:
            for i in range(TK):
                xs = xst.tile([128, C], F32)
                nc.sync.dma_start(out=xs, in_=x_d[ts(i, 128), :])
                for j in range(CK):
                    pt = ptr.tile([128, 128], F32)
                    nc.tensor.transpose(pt, xs[:, ts(j, 128)], ident)
                    nc.vector.tensor_copy(xT[:, j, ts(i, 128)], pt)

        # ---- phase 2: qk^T (transposed) and v (standard, with ones col) ----
        # Emission order interleaves q/k tile pairs (head-pair h//2 needs
        # qkT tiles i and 4+i) with v t-tiles so head-0 attention can start
        # after ~1/4 of this phase instead of all of it.
        nc.gpsimd.memset(vaug[:, :, :, D:D + 1], 1.0)
        with tc.tile_pool(name="pqk", bufs=4, space="PSUM") as pqk:

            def qk_tile(i):
                for n in range(T // 512):          # 512-chunk of t
                    ps = pqk.tile([128, 512], F32)
                    for j in range(CK):
                        nc.tensor.matmul(
                            ps, wqk_s[:, j, ts(i, 128)],
                            xT[:, j, ts(n, 512)],
                            start=(j == 0), stop=(j == CK - 1))
                    nc.vector.tensor_scalar_add(qkT[:, i, ts(n, 512)], ps,
                                                bqk_s[:, i:i + 1])

            def v_tile(i):
                ps = pqk.tile([128, 512], F32)
                for j in range(CK):
                    nc.tensor.matmul(
                        ps, xT[:, j, ts(i, 128)], wv_s[:, j, :],
                        start=(j == 0), stop=(j == CK - 1))
                nc.scalar.activation(
                    vaug[:, i, :, 0:D],
                    ps.rearrange("p (h d) -> p h d", h=H), AF.Copy)

            for i in range(TK):
                v_tile(i)
            for i in range(CK):
                qk_tile(i)           # q tiles for head-pair i
                qk_tile(CK + i)      # k tiles for head-pair i

        # ---- phase 3: attention per head ----
        if upto < 3:
            return
        with tc.tile_pool(name="ps_s", bufs=2, space="PSUM") as ps_s, \
             tc.tile_pool(name="ps_y", bufs=2, space="PSUM") as ps_y, \
             tc.tile_pool(name="expp", bufs=6) as expp, \
             tc.tile_pool(name="rp", bufs=4) as rp, \
             tc.tile_pool(name="rbp", bufs=4) as rbp:
            for h in range(H):
                po = (h % 2) * 64
                qt = h // 2
                kt_ = C // 128 + h // 2
                y_ps = ps_y.tile([D + 1, T], F32)
                for kt in range(TK):
                    s_ps = ps_s.tile([128, T], F32)
                    for n in range(T // 512):
                        nc.tensor.matmul(
                            s_ps[:, ts(n, 512)],
                            qkT[po:po + D, kt_, ts(kt, 128)],
                            qkT[po:po + D, qt, ts(n, 512)],
                            start=True, stop=True)
                    et = expp.tile([128, T], BF16)
                    nc.scalar.activation(et, s_ps, AF.Exp)
                    # columns q >= 128*(kt+1)-1 are fully unmasked for this
                    # k-tile (causal boundary passed), so only multiply the
                    # masked prefix
                    mw = min(T, 128 * (kt + 1))
                    nc.vector.tensor_mul(et[:, :mw], et[:, :mw],
                                         mask_s[:, kt, :mw])
                    for n in range(T // 512):
                        nc.tensor.matmul(
                            y_ps[:, ts(n, 512)], vaug[:, kt, h, :],
                            et[:, ts(n, 512)],
                            start=(kt == 0), stop=(kt == TK - 1))
                rec = rp.tile([1, T], F32)
                nc.vector.reciprocal(rec, y_ps[D:D + 1, :])
                rb = rbp.tile([D, T], F32)
                nc.gpsimd.partition_broadcast(rb, rec)
                nc.vector.tensor_mul(yT[po:po + D, qt, :], y_ps[0:D, :], rb)

        # ---- phase 4: out = y @ W_proj + b_eff ----
        if upto < 4:
            return
        with tc.tile_pool(name="pp", bufs=2, space="PSUM") as pp, \
             tc.tile_pool(name="outst", bufs=3) as outst:
            for i in range(TK):
                ps = pp.tile([128, C], F32)
                nc.tensor.matmul(ps, ones1, beff_s,
                                 start=True, stop=False)
                for j in range(CK):
                    nc.tensor.matmul(ps, yT[:, j, ts(i, 128)],
                                     wp_s[:, j, :],
                                     start=False, stop=(j == CK - 1))
                ot = outst.tile([128, C], F32)
                nc.scalar.copy(ot, ps)
                nc.sync.dma_start(out=out_d[ts(i, 128), :], in_=ot)


def get_nc(reps=1, upto=4):
    key = ("nc", reps, upto)
    if key not in _CACHE:
        _CACHE[key] = _build_nc(reps, upto)
    return _CACHE[key]


def tf32_round(a):
    """Round-to-nearest-even to tf32 (10-bit mantissa). fp32r operands must be
    pre-rounded: the BIR verifier requires every producer of fp32r-matmul
    operands to emit rounded values, and DMA can't convert."""
    a = np.ascontiguousarray(a, np.float32)
    b = a.view(np.uint32)
    lsb = (b >> np.uint32(13)) & np.uint32(1)
    r = b + np.uint32(0x0FFF) + lsb
    return ((r >> np.uint32(13)) << np.uint32(13)).view(np.float32)


def make_in_maps(x, padding_mask, W_qkv, b_qkv, W_proj, b_proj):
    x = np.asarray(x, np.float32)
    padding_mask = np.asarray(padding_mask, bool)
    W_qkv = np.asarray(W_qkv, np.float32)
    b_qkv = np.asarray(b_qkv, np.float32)
    W_proj = np.asarray(W_proj, np.float32)
    b_proj = np.asarray(b_proj, np.float32)

    scale = np.float32(1.0 / np.sqrt(D))
    wqk = np.concatenate([W_qkv[:, :C] * scale, W_qkv[:, C:2 * C]], axis=1)
    wqk = tf32_round(wqk)
    wv = tf32_round(W_qkv[:, 2 * C:])
    wp = tf32_round(W_proj)
    bqk = np.concatenate([b_qkv[:C] * scale, b_qkv[C:2 * C]]).astype(np.float32)
    beff = tf32_round((b_qkv[2 * C:] @ W_proj + b_proj).reshape(1, C))

    kidx = np.arange(T, dtype=np.int32)[:, None]
    qidx = np.arange(T, dtype=np.int32)[None, :]
    causalT = kidx <= qidx                      # [k, q]
    maskT = (causalT[None] | padding_mask[:, None, :])  # [TT, k, q]
    maskT = maskT.astype(ml_dtypes.bfloat16)

    in_maps = []
    for b in range(B):
        in_maps.append({
            "x": np.ascontiguousarray(x[b]),
            "maskT": np.ascontiguousarray(maskT[b % TT]),
            "wqk": wqk, "wv": wv, "wp": wp,
            "bqk": bqk, "beff": beff,
            "ones1": np.ones((1, 128), np.float32),
        })
    return in_maps


def kernel(x, padding_mask, W_qkv, b_qkv, W_proj, b_proj):
    from concourse.bass_utils import run_bass_kernel_spmd

    nc = get_nc()
    in_maps = make_in_maps(x, padding_mask, W_qkv, b_qkv, W_proj, b_proj)
    res = run_bass_kernel_spmd(nc, in_maps, list(range(NCORES)))
    out = np.stack([res.results[b]["out"] for b in range(B)])
    return out.astype(np.float32)
